# revision 22
# baseline (speedup 1.0000x reference)
"""Causal attention (B=4, S=4096, D_IN=768, D_OUT=64) on 8 Trainium2 NeuronCores.

Sharding: core c handles batch b=c//2 and key-parity p=c%2 (the even or odd
128-wide key tiles of that batch). Every core computes, for ALL queries of its
batch, the unnormalized attention partials over its own key set:
    num[o, q] = sum_{k in own} exp(q.k/8) * V[k, o]
    den[q]    = sum_{k in own} exp(q.k/8)
The host sums the two partials per batch and normalizes: ctx = (num/den).T.
Causality is exact: key-tile work is skipped below the diagonal band and the
boundary blocks are masked with host-provided mask tiles.

Schedule (all bf16 on-chip, fp32 PSUM). Two serial resources matter:
the Scalar/ACT engine (~34us of exp streaming at 1.2GHz) and the PE
(~36us of matmul streaming at 2.4GHz -- but only 1.2GHz unless it has
run gap-free for 3us, so every stall is paid twice). The schedule:
 - x arrives via 4 DMA rings (scalar/gpsimd/vector/sync) with block 0 as
   two parallel halves and the weights split [KV|QQ] so the first
   projection matmul waits only on twKV + half 0.
 - dummy matmuls bridge the PE from program start to the first data
   arrival so the p-state ramp is complete when real work starts.
 - block-0 projections run per half as each half lands; kp/vts/qts
   copies are split between Vector and GpSimd.
 - the main loop emits one scores-group + one exp per "slot" and uses a
   filler queue (ctx drains of the previous tile, projection units for
   upcoming blocks) to pad the PE between slots, so the ACT stream
   never waits and the PE never idles.
 - scores matmuls run as CONCURRENT K=64 pairs on disjoint PE row
   groups (kp[i][0:64] = K^T of key tile 2i, kp[i][64:128] = tile 2i+1;
   Wq is sent duplicated so Q^T exists at partitions 0..63 AND 64..127).
 - ctx matmuls stay M=65 (64 V columns + ones column -> denominator
   free).
 - the last tile's ctx is drained in two column halves so the output
   copy+DMA of the first half overlaps the second half's matmuls.
"""
import numpy as np

import concourse.bass as bass
import concourse.bacc as bacc
import concourse.tile as tile
from concourse import mybir
from concourse.bass_utils import run_bass_kernel_spmd

B, S, DI, DO = 4, 4096, 768, 64
NCORES = 8
NIC = DI // 128          # 6 contraction chunks
NKT = S // 128           # 32 global key tiles per batch
NOWN = NKT // 2          # 16 own key tiles per core
QT = 512                 # query tile width
NQT = S // QT            # 8 query tiles
ORD = [0, 1, 3, 2, 7, 6, 5, 4]       # query-tile processing order
F32 = mybir.dt.float32
BF16 = mybir.dt.bfloat16
F8 = mybir.dt.float8e4
NWARM = 44               # dummy warmup matmuls (PE p-state bridge)

_prog_cache = {}


def j0_of(T):
    """First diagonal-region packed key tile for permuted query tile T."""
    return 4 * T if T < 4 else 4 * (T - 4)


def build_program():
    """Build + compile the single SPMD Bass program (identical on all cores)."""
    nc = bacc.Bacc("TRN2", target_bir_lowering=False, debug=False)

    # x^T relaid by the host to [partition, block, chunk, col]; block 0 is
    # stored [p, half, chunk, 256] so each half is one contiguous DMA.  Only
    # blocks 0-3 (own keys) are needed in bf16 (K/V projections); the Q
    # projection for blocks 1-7 runs in fp8 e4m3 DoubleRow (2x PE rate), fed
    # by a separate fp8 copy of x.
    xT = nc.declare_dram_parameter("xT", [128, 4 * NIC * QT], BF16,
                                   isOutput=False)
    xT8 = nc.declare_dram_parameter("xT8", [128, 7 * NIC * QT], F8,
                                    isOutput=False)
    # [Wk|Wv] then [Wq|Wq] (Wq duplicated so Q^T appears at partitions 0..63
    # AND 64..127), each relaid to [128, chunk, 128] contiguous per partition.
    wall = nc.declare_dram_parameter("wall", [128, 2 * NIC * 128], BF16,
                                     isOutput=False)
    w8 = nc.declare_dram_parameter("w8", [128, NIC * 128], F8, isOutput=False)
    # [mdiag | mpcol | ident(zero-padded)] as one [128, 320] block
    mall = nc.declare_dram_parameter("mall", [128, 320], BF16, isOutput=False)
    nd = nc.declare_dram_parameter("nd", [DO + 1, S], BF16, isOutput=True)

    with tile.TileContext(nc) as tc:
        with tc.tile_pool(name="consts", bufs=1) as consts, \
             tc.tile_pool(name="xpool", bufs=1) as xpool, \
             tc.tile_pool(name="qkv", bufs=1) as qkv, \
             tc.tile_pool(name="expp", bufs=10) as expp, \
             tc.tile_pool(name="ndst", bufs=2) as ndst, \
             tc.tile_pool(name="ps_sc", bufs=2, space="PSUM") as ps_sc, \
             tc.tile_pool(name="ps_pj", bufs=1, space="PSUM") as ps_pj, \
             tc.tile_pool(name="ps_ctx", bufs=1, space="PSUM") as ps_ctx:

            BW = NIC * QT  # 3072 cols per x block
            HB = BW // 2
            # ---- input DMAs, issued first thing.  Only sync/scalar/gpsimd
            # queues can start DMAs, and the sync/SP ring is ~10x slower than
            # the other two, so all bulk data rides scalar + gpsimd:
            #   scalar: xb0 half0, xb0 half1 (then free for the exp stream)
            #   gpsimd: twKV, twQ, [gate on h1], xb1, xb3, xb2, xb7, xb6,
            #           xb5, xb4  (+ nd outputs later)
            #   sync:   tm (small, not urgent)
            # The gate keeps xb1 from stealing HBM bandwidth from block 0.
            xb = [None] + [xpool.tile([128, BW], BF16, tag=f"xb_{cb}",
                                      name=f"xb_{cb}")
                           for cb in range(1, 4)]
            xb8 = [None] + [xpool.tile([128, NIC, QT], F8, tag=f"xb8_{cb}",
                                       name=f"xb8_{cb}")
                            for cb in range(1, 4)]
            xb8t = xpool.tile([128, 4, NIC, QT], F8, tag="xb8t", name="xb8t")
            xb8 += [xb8t[:, cb - 4] for cb in range(4, NQT)]
            xb0h = [xpool.tile([128, HB], BF16, tag=f"xb0h{h}", name=f"xb0h{h}")
                    for h in range(2)]
            twKV = consts.tile([128, NIC, 128], BF16, tag="twKV", name="twKV")
            twQ = consts.tile([128, NIC, 128], BF16, tag="twQ", name="twQ")
            tm = consts.tile([128, 320], BF16, tag="tm", name="tm")

            tw8 = consts.tile([128, NIC, 128], F8, tag="tw8", name="tw8")
            # Descriptor generation costs ~0.65us per DMA on the issuing
            # queue, so the 15 input DMAs are split: block 0 + block 1 ride
            # the scalar queue (free until the exp stream), the rest ride
            # gpsimd, both in consumption order.
            # The scalar ring gets ~2x bandwidth priority early, so block 0
            # rides it; weights lead the gpsimd ring.
            nc.gpsimd.dma_start(out=twKV, in_=wall[:, 0:NIC * 128])
            nc.scalar.dma_start(out=xb0h[0], in_=xT[:, 0:HB])
            nc.scalar.dma_start(out=xb0h[1], in_=xT[:, HB:BW])
            nc.gpsimd.dma_start(out=twQ, in_=wall[:, NIC * 128:2 * NIC * 128])
            nc.gpsimd.dma_start(out=tw8, in_=w8[:, :])
            nc.scalar.dma_start(out=xb8[1], in_=xT8[:, 0:BW])
            nc.scalar.dma_start(out=xb[1], in_=xT[:, BW:2 * BW])
            nc.sync.dma_start(out=tm, in_=mall[:, :])
            nc.gpsimd.dma_start(out=xb8[3], in_=xT8[:, 2 * BW:3 * BW])
            nc.gpsimd.dma_start(out=xb[2], in_=xT[:, 2 * BW:3 * BW])
            nc.gpsimd.dma_start(out=xb[3], in_=xT[:, 3 * BW:4 * BW])
            nc.gpsimd.dma_start(out=xb8[2], in_=xT8[:, 1 * BW:2 * BW])
            # blocks 4-7 (fp8 only, Q-proj) as ONE descriptor: descriptor
            # generation costs ~0.65us each on the issuing queue.
            nc.gpsimd.dma_start(out=xb8t, in_=xT8[:, 3 * BW:7 * BW])

            tmd = tm[:, 0:128]
            tmp = tm[:, 128:256]
            tid = tm[0:DO, 256:320]

            # ---- PE p-state bridge: dummy matmuls from program start until
            # the first x data lands, so the 3us continuous-execution ramp is
            # complete when real work starts.
            dum = consts.tile([128, 128], BF16, tag="dum", name="dum")
            nc.vector.memset(dum, 0.0)
            pdum = ps_sc.tile([128, 3 * QT], F32, tag="psc", name="psc")
            for _ in range(NWARM):
                nc.tensor.matmul(pdum[:, 0:128], dum, dum, start=True, stop=True)

            zsrc = consts.tile([DO, 1], F32, tag="zsrc", name="zsrc")
            nc.vector.memset(zsrc, 0.0)
            # Dummy exp pulls the ~1.3us ACT table load off the critical path.
            zexp = consts.tile([DO, 1], F32, tag="zexp", name="zexp")
            nc.scalar.activation(zexp, zsrc,
                                 mybir.ActivationFunctionType.Exp, scale=1.0)

            def xc(ic, cb):
                return xb[cb][:, ic * QT:(ic + 1) * QT]

            def xc0(half, ic):
                return xb0h[half][:, ic * 256:(ic + 1) * 256]

            # ---- projection state ----
            # kp[i]: K^T of key tile 2i at partitions 0..63, tile 2i+1 at
            # 64..127
            kps = [qkv.tile([128, 128], BF16, tag=f"kp_{i}", name=f"kp_{i}")
                   for i in range(NOWN // 2)]
            vts = [qkv.tile([DO, QT], BF16, tag=f"vt_{st}", name=f"vt_{st}")
                   for st in range(4)]
            qts = [qkv.tile([128, QT], BF16, tag=f"qt_{st}", name=f"qt_{st}")
                   for st in range(NQT)]
            # all V1 tiles in one buffer: [128 keys, key tile, 64 V cols + 1s]
            v1big = qkv.tile([128, NOWN, DO + 1], BF16, tag="v1big",
                             name="v1big")
            nc.vector.memset(v1big[:, :, DO:DO + 1], 1.0)

            def v1(j):
                return v1big[:, j, :]

            def kv_units(st):
                """K/V projection of own key column block st, as small PE
                units; copies split across Vector and GpSimd."""
                p1 = ps_pj.tile([128, QT], F32, tag="pspj", name="pspj")
                for ic in range(0, NIC, 2):
                    def mm2(ic=ic, p1=p1):
                        nc.tensor.matmul(p1, twKV[:, ic, :], xc(ic, st),
                                         start=(ic == 0), stop=False)
                        nc.tensor.matmul(p1, twKV[:, ic + 1, :], xc(ic + 1, st),
                                         start=False, stop=(ic + 1 == NIC - 1))
                    yield mm2

                def copies(p1=p1):
                    nc.vector.tensor_copy(vts[st], p1[DO:128, :])
                    for u in range(2):
                        kp = kps[2 * st + u]
                        nc.vector.tensor_copy(kp[0:DO, :],
                                              p1[0:DO, 256 * u:256 * u + 128])
                        nc.vector.tensor_copy(kp[DO:128, :],
                                              p1[0:DO, 256 * u + 128:256 * u + 256])
                yield copies

            def tr_unit(st, pre=None):
                """V transposes for block st -> v1big rows 4st..4st+3."""
                if pre is not None:
                    yield pre
                def transp():
                    pvq = ps_pj.tile([128, 4, DO], BF16, tag="pspj", name="pspj")
                    for r in range(4):
                        nc.tensor.transpose(pvq[:, r, :],
                                            vts[st][:, r * 128:r * 128 + 128],
                                            tid)
                    nc.vector.tensor_copy(v1big[:, 4 * st:4 * st + 4, 0:DO], pvq)
                yield transp

            def q_units(st):
                """Q^T (duplicated at partitions 0..63 / 64..127) for block
                st, in fp8 e4m3 DoubleRow mode: each matmul contracts TWO
                128-chunks at once at 2x PE rate."""
                p2 = ps_pj.tile([128, QT], F32, tag="pspj", name="pspj")
                for k in range(NIC // 2):
                    def mm(k=k, p2=p2):
                        nc.tensor.matmul(
                            p2, tw8[:, 2 * k:2 * k + 2, :],
                            xb8[st][:, 2 * k:2 * k + 2, :],
                            start=(k == 0), stop=(k == NIC // 2 - 1),
                            perf_mode=mybir.MatmulPerfMode.DoubleRow)
                    yield mm

                def qcopy(p2=p2):
                    nc.vector.tensor_copy(qts[st], p2)
                yield qcopy

            exp_scale = float(1.0 / np.sqrt(DO))

            def mm_sc(T, j, w, sp, off):
                """One K=64 scores matmul: key tile j x last w queries of tile
                T, into sp[:, off:off+w]. Row-group from j's parity."""
                kp = kps[j // 2]
                lo = DO * (j % 2)
                nc.tensor.matmul(sp[:, off:off + w], kp[lo:lo + DO, :],
                                 qts[T][lo:lo + DO, QT - w:QT],
                                 start=True, stop=True)

            class CtxDrain:
                """Phase B for a query tile, drained a few matmuls at a time
                via the filler queue so ctx work interleaves between the next
                tile's scores groups in the in-order PE queue."""

                def __init__(self, T, ctx_args):
                    self.T = T
                    self.nk = j0_of(T) + 4
                    self.args = ctx_args
                    self.i = 0
                    self.ctxp = ps_ctx.tile([DO + 1, QT], F32, tag="ctxp",
                                            name="ctxp")

                def drain(self, n):
                    while self.i < len(self.args) and n > 0:
                        j, et_ap, qlo, w = self.args[self.i]
                        nc.tensor.matmul(self.ctxp[:, qlo:QT], v1(j),
                                         et_ap[:, 0:w],
                                         start=(j == 0), stop=(j == self.nk - 1))
                        self.i += 1
                        n -= 1

                def finish(self):
                    self.drain(len(self.args))
                    ost = ndst.tile([DO + 1, QT], BF16, tag="ost", name="ost")
                    nc.vector.tensor_copy(ost, self.ctxp)
                    nc.gpsimd.dma_start(out=nd[:, self.T * QT:(self.T + 1) * QT],
                                        in_=ost)

            def emit_scores_full(T, j, cnt):
                sp = ps_sc.tile([128, 3 * QT], F32, tag="psc", name="psc")
                et = expp.tile([128, 3 * QT], BF16, tag="et", name="et")
                for u in range(cnt):
                    mm_sc(T, j + u, QT, sp, u * QT)
                return (sp, et, j, cnt)

            def emit_scores_band(T, j0):
                # diagonal band: all 4 tiles in ONE 3-bank tile / one exp:
                # r0 [0:512] bank1, r1 [512:896] bank2, r3 [896:1024] bank2,
                # r2 [1024:1280] bank3 (concurrent pairs hit distinct banks).
                sp = ps_sc.tile([128, 3 * QT], F32, tag="psc", name="psc")
                et = expp.tile([128, 3 * QT], BF16, tag="et", name="et")
                mm_sc(T, j0, QT, sp, 0)
                mm_sc(T, j0 + 1, 384, sp, QT)
                mm_sc(T, j0 + 2, 256, sp, 2 * QT)
                mm_sc(T, j0 + 3, 128, sp, QT + 384)
                return (sp, et, j0, -1)

            # ---- block-0 projections, per half: each half's matmul chain,
            # then its kp/vts/qts copies, start as soon as that half lands.
            p1 = ps_pj.tile([128, QT], F32, tag="pspj", name="pspj")
            p2 = ps_sc.tile([128, 3 * QT], F32, tag="psc", name="psc")
            for half in range(2):
                for ic in range(NIC):
                    nc.tensor.matmul(p1[:, half * 256:half * 256 + 256],
                                     twKV[:, ic, :], xc0(half, ic),
                                     start=(ic == 0), stop=(ic == NIC - 1))
                for ic in range(NIC):
                    nc.tensor.matmul(p2[:, half * 256:half * 256 + 256],
                                     twQ[:, ic, :], xc0(half, ic),
                                     start=(ic == 0), stop=(ic == NIC - 1))
                kp = kps[half]
                nc.vector.tensor_copy(kp[0:DO, :],
                                      p1[0:DO, 256 * half:256 * half + 128])
                nc.vector.tensor_copy(
                    kp[DO:128, :],
                    p1[0:DO, 256 * half + 128:256 * half + 256])
                nc.vector.tensor_copy(qts[0][:, 256 * half:256 * half + 256],
                                      p2[:, 256 * half:256 * half + 256])

            def b0_vts_copies():
                # vts[0] casts deferred off the band critical path: only the
                # tr0 transposes (pre first T0-ctx drain) need them.
                for half in range(2):
                    nc.vector.tensor_copy(
                        vts[0][:, 256 * half:256 * half + 256],
                        p1[DO:128, 256 * half:256 * half + 256])

            # ---- main loop ----
            # fillers: list of (deadline, seq, closure) proj units, kept
            # sorted (stable) by deadline.  Deadlines are GLOBAL SLOT ids:
            # each scores-group emission is one slot, numbered across the
            # whole kernel; a unit with deadline s is flushed before slot
            # s's scores are emitted.  Units of one generator share a
            # deadline, so stable sorting keeps each accumulation chain
            # contiguous in emission order (they share one PSUM buffer).
            fillers = []
            _seq = [0]

            def push_units(gen, dl):
                for u in gen:
                    fillers.append((dl, _seq[0], u))
                    _seq[0] += 1
                fillers.sort(key=lambda t: (t[0], t[1]))

            def run_fillers(n):
                k = 0
                while fillers and k < n:
                    fillers.pop(0)[2]()
                    k += 1

            def flush(s):
                while fillers and fillers[0][0] <= s:
                    fillers.pop(0)[2]()

            # slots per position: ceil(j0/3) full groups + 1 band
            SLOTS = [-(-j0_of(t) // 3) + 1 for t in ORD]
            START = [sum(SLOTS[:p]) for p in range(NQT)]   # first slot id
            BAND = [START[p] + SLOTS[p] - 1 for p in range(NQT)]

            # projection units per position: (st, kind, slot deadline).
            PROJ = {
                0: [(1, "q", START[1]), (0, "tr0", START[1]),
                    (1, "kv", BAND[1]), (1, "tr", START[2] + 2)],
                1: [(3, "q", START[2]), (2, "kv", START[2] + 2),
                    (3, "kv", BAND[2])],
                2: [(2, "q", START[3]), (2, "tr", START[3] + 1),
                    (3, "tr", START[3] + 2)],
                3: [(7, "q", START[4])],
                4: [(6, "q", START[5])],
                5: [(5, "q", START[6])],
                6: [(4, "q", START[7])],
                7: [],
            }

            pending = None  # CtxDrain from the previous iteration
            pre = None      # scores group already emitted via lookahead
            for pos in range(NQT):
                T = ORD[pos]
                j0 = j0_of(T)
                mask = tmd if T < 4 else tmp
                ctx_args = []   # (j, et_ap, qlo, w) drained via pending

                for st, kind, dl in PROJ[pos]:
                    if kind == "q":
                        push_units(q_units(st), dl)
                    elif kind == "kv":
                        push_units(kv_units(st), dl)
                    elif kind == "tr0":
                        push_units(tr_unit(st, pre=b0_vts_copies), dl)
                    else:
                        push_units(tr_unit(st), dl)
                # correctness: everything slot START[pos] (this position's
                # g0 scores / first ctx drains) depends on must be emitted.
                flush(START[pos])

                # group descriptors: (j, cnt) fulls in triples, then band
                descs = [(j, min(3, j0 - j)) for j in range(0, j0, 3)]
                descs.append((j0, -1))

                # pace leftovers + prev tile's ctx across the full-tile slots
                nslots = len(descs) - 1
                per_slot = -(-len(fillers) // nslots) if nslots else 0
                dn = (-(-len(pending.args) // nslots)
                      if pending is not None and nslots else 0)

                for gi, (j, cnt) in enumerate(descs):
                    is_band = cnt < 0
                    if gi == 0:
                        g = pre if pre is not None else (
                            emit_scores_band(T, j) if is_band
                            else emit_scores_full(T, j, cnt))
                        pre = None
                    # exp for this group
                    sp, et = g[0], g[1]
                    if is_band:
                        nc.scalar.activation(
                            et[:, 0:2 * QT + 256], sp[:, 0:2 * QT + 256],
                            mybir.ActivationFunctionType.Exp, scale=exp_scale)
                    else:
                        nc.scalar.activation(
                            et[:, 0:cnt * QT], sp[:, 0:cnt * QT],
                            mybir.ActivationFunctionType.Exp, scale=exp_scale)
                        for u in range(cnt):
                            ctx_args.append((j + u, et[:, u * QT:(u + 1) * QT],
                                             0, QT))
                    # lookahead: emit the NEXT group's scores now (after
                    # flushing exactly the units that group depends on), so
                    # the PE filler work of this slot can never starve the
                    # ACT stream.
                    if gi + 1 < len(descs):
                        jn, cn = descs[gi + 1]
                        flush(START[pos] + gi + 1)
                        if cn < 0:
                            g = emit_scores_band(T, jn)
                        else:
                            g = emit_scores_full(T, jn, cn)
                    elif pos < NQT - 1:
                        # boundary: next position's g0, after its deps
                        flush(START[pos + 1])
                        Tn = ORD[pos + 1]
                        j0n = j0_of(Tn)
                        pre = (emit_scores_band(Tn, j0n) if j0n == 0
                               else emit_scores_full(Tn, 0, min(3, j0n)))

                    if is_band:
                        # masks: split across Vector and GpSimd mid-kernel;
                        # Vector at the edges (gpsimd queue busy with DMA
                        # descriptors early; the tail ctx gates on them at
                        # the end).  Half-A deps (cols 0:128, 512:640) first.
                        meng = nc.vector if pos == NQT - 1 else nc.gpsimd
                        nc.vector.tensor_mul(et[:, 0:128], et[:, 0:128], mask)
                        meng.tensor_mul(et[:, QT:QT + 128],
                                        et[:, QT:QT + 128], mask)
                        nc.vector.tensor_mul(et[:, QT + 384:2 * QT],
                                             et[:, QT + 384:2 * QT], mask)
                        meng.tensor_mul(et[:, 2 * QT:2 * QT + 128],
                                        et[:, 2 * QT:2 * QT + 128], mask)
                        ctx_args.append((j0, et[:, 0:QT], 0, QT))
                        ctx_args.append((j0 + 1, et[:, QT:QT + 384], 128, 384))
                        ctx_args.append((j0 + 2, et[:, 2 * QT:2 * QT + 256],
                                         256, 256))
                        ctx_args.append((j0 + 3, et[:, QT + 384:2 * QT],
                                         384, 128))
                        if pending is not None:
                            pending.finish()
                    else:
                        if pending is not None:
                            pending.drain(dn)
                        run_fillers(per_slot)

                if pos < NQT - 1:
                    pending = CtxDrain(T, ctx_args)
                else:
                    # ---- tail: drain the last tile's ctx in two column
                    # halves; the two output DMAs ride different rings so
                    # they stream in parallel.  j0 == 0 (band only).
                    run_fillers(len(fillers))
                    eb = et  # band et tile (last group processed)
                    ctxp = ps_ctx.tile([DO + 1, QT], F32, tag="ctxp",
                                       name="ctxp")
                    H = QT // 2
                    # half A: output cols 0:256 <- tiles j0 (cols 0:256) and
                    # j0+1 (out cols 128:256 = its et cols 0:128)
                    nc.tensor.matmul(ctxp[:, 0:H], v1(0), eb[:, 0:H],
                                     start=True, stop=False)
                    nc.tensor.matmul(ctxp[:, 128:H], v1(1), eb[:, QT:QT + 128],
                                     start=False, stop=True)
                    ostA = ndst.tile([DO + 1, H], BF16, tag="ost", name="ostA")
                    nc.vector.tensor_copy(ostA, ctxp[:, 0:H])
                    nc.scalar.dma_start(out=nd[:, T * QT:T * QT + H],
                                        in_=ostA)
                    # half B: output cols 256:512
                    ctxp2 = ps_pj.tile([DO + 1, H], F32, tag="pspj",
                                       name="ctxp2")
                    nc.tensor.matmul(ctxp2, v1(0), eb[:, H:QT],
                                     start=True, stop=False)
                    nc.tensor.matmul(ctxp2[:, 0:H], v1(1),
                                     eb[:, QT + 128:QT + 384],
                                     start=False, stop=False)
                    nc.tensor.matmul(ctxp2[:, 0:H], v1(2),
                                     eb[:, 2 * QT:2 * QT + 256],
                                     start=False, stop=False)
                    nc.tensor.matmul(ctxp2[:, 128:H], v1(3),
                                     eb[:, QT + 384:2 * QT],
                                     start=False, stop=True)
                    ostB = ndst.tile([DO + 1, H], BF16, tag="ost", name="ostB")
                    nc.vector.tensor_copy(ostB, ctxp2)
                    nc.gpsimd.dma_start(out=nd[:, T * QT + H:(T + 1) * QT],
                                        in_=ostB)

    nc.compile()
    return nc


def get_program():
    if "nc" not in _prog_cache:
        _prog_cache["nc"] = build_program()
    return _prog_cache["nc"]


def core_perm(parity):
    """Permuted-to-global column index map: own key tiles first, then other."""
    own = [g for g in range(NKT) if g % 2 == parity]
    other = [g for g in range(NKT) if g % 2 != parity]
    return np.concatenate([np.arange(g * 128, (g + 1) * 128)
                           for g in own + other])


def _to_bf16(a):
    from concourse import mybir as _mybir
    return np.ascontiguousarray(a.astype(_mybir.dt.np(_mybir.dt.bfloat16)))


def _to_f8(a):
    from concourse import mybir as _mybir
    return np.ascontiguousarray(a.astype(_mybir.dt.np(_mybir.dt.float8e4)))


def make_in_maps(x, Wq, Wk, Wv):
    x = np.asarray(x, dtype=np.float32)
    Wq = np.asarray(Wq, dtype=np.float32)
    Wk = np.asarray(Wk, dtype=np.float32)
    Wv = np.asarray(Wv, dtype=np.float32)
    wkv = np.concatenate([Wk, Wv], axis=1)                     # [768, 128]
    wqq = np.concatenate([Wq, Wq], axis=1)                     # [768, 128]
    wkv_r = wkv.reshape(NIC, 128, 128).transpose(1, 0, 2).reshape(128, -1)
    wqq_r = wqq.reshape(NIC, 128, 128).transpose(1, 0, 2).reshape(128, -1)
    wall = _to_bf16(np.concatenate([wkv_r, wqq_r], axis=1))    # [128, 1536]
    w8 = _to_f8(wqq_r)                                         # [128, 768]
    mdiag = np.triu(np.ones((128, 128), dtype=np.float32))
    identp = np.concatenate([np.eye(DO, dtype=np.float32),
                             np.zeros((128 - DO, DO), np.float32)], axis=0)
    in_maps = []
    perms = []
    for c in range(NCORES):
        b, par = c // 2, c % 2
        perm = core_perm(par)
        perms.append(perm)
        xTp = x[b].T[:, perm]                                  # [768, 4096]
        # [p, block, chunk, col] layout, contiguous per partition per block;
        # block 0 is stored [p, half, chunk, 256] so its two column halves
        # are each one contiguous DMA
        blocks = xTp.reshape(NIC, 128, NQT, QT).transpose(1, 2, 0, 3)
        b0 = (blocks[:, 0].reshape(128, NIC, 2, 256).transpose(0, 2, 1, 3)
              .reshape(128, NIC * QT))
        rest = blocks[:, 1:4].reshape(128, 3 * NIC * QT)
        xr = np.concatenate([b0, rest], axis=1)                # bf16: blk 0-3
        x8r = blocks[:, 1:].reshape(128, (NQT - 1) * NIC * QT)  # fp8: blk 1-7
        mpcol = np.full((128, 128), 1.0 - par, dtype=np.float32)
        mall = np.concatenate([mdiag, mpcol, identp], axis=1)  # [128, 320]
        in_maps.append({
            "xT": _to_bf16(xr), "xT8": _to_f8(x8r), "wall": wall, "w8": w8,
            "mall": _to_bf16(mall),
        })
    return in_maps, perms


def combine(results, perms):
    out = np.empty((B, S, DO), dtype=np.float32)
    for b in range(B):
        num = np.zeros((DO, S), dtype=np.float64)
        den = np.zeros((S,), dtype=np.float64)
        for c in (2 * b, 2 * b + 1):
            nd_c = results[c]["nd"].astype(np.float64)
            inv = np.empty(S, dtype=np.int64)
            inv[perms[c]] = np.arange(S)
            nd_g = nd_c[:, inv]
            num += nd_g[:DO]
            den += nd_g[DO]
        out[b] = (num / den).T.astype(np.float32)
    return out


def kernel(x, Wq, Wk, Wv):
    nc = get_program()
    in_maps, perms = make_in_maps(x, Wq, Wk, Wv)
    res = run_bass_kernel_spmd(nc, in_maps, list(range(NCORES)))
    return combine(res.results, perms)


# revision 23
# speedup vs baseline: 1.1030x; 1.1030x over previous
"""Causal attention (B=4, S=4096, D_IN=768, D_OUT=64) on 8 Trainium2 NeuronCores.

Sharding: core c handles batch b=c//2 and key-parity p=c%2 (the even or odd
128-wide key tiles of that batch). Every core computes, for ALL queries of its
batch, the unnormalized attention partials over its own key set:
    num[o, q] = sum_{k in own} exp(q.k/8) * V[k, o]
    den[q]    = sum_{k in own} exp(q.k/8)
The host sums the two partials per batch and normalizes: ctx = (num/den).T.
Causality is exact: key-tile work is skipped below the diagonal band and the
boundary blocks are masked with host-provided mask tiles.

Schedule (all bf16 on-chip, fp32 PSUM). Two serial resources matter:
the Scalar/ACT engine (~34us of exp streaming at 1.2GHz) and the PE
(~36us of matmul streaming at 2.4GHz -- but only 1.2GHz unless it has
run gap-free for 3us, so every stall is paid twice). The schedule:
 - x arrives via 4 DMA rings (scalar/gpsimd/vector/sync) with block 0 as
   two parallel halves and the weights split [KV|QQ] so the first
   projection matmul waits only on twKV + half 0.
 - dummy matmuls bridge the PE from program start to the first data
   arrival so the p-state ramp is complete when real work starts.
 - block-0 projections run per half as each half lands; kp/vts/qts
   copies are split between Vector and GpSimd.
 - the main loop emits one scores-group + one exp per "slot" and uses a
   filler queue (ctx drains of the previous tile, projection units for
   upcoming blocks) to pad the PE between slots, so the ACT stream
   never waits and the PE never idles.
 - scores matmuls run as CONCURRENT K=64 pairs on disjoint PE row
   groups (kp[i][0:64] = K^T of key tile 2i, kp[i][64:128] = tile 2i+1;
   Wq is sent duplicated so Q^T exists at partitions 0..63 AND 64..127).
 - ctx matmuls stay M=65 (64 V columns + ones column -> denominator
   free).
 - the last tile's ctx is drained in two column halves so the output
   copy+DMA of the first half overlaps the second half's matmuls.
"""
import numpy as np

import concourse.bass as bass
import concourse.bacc as bacc
import concourse.tile as tile
from concourse import mybir
from concourse.bass_utils import run_bass_kernel_spmd

B, S, DI, DO = 4, 4096, 768, 64
NCORES = 8
NIC = DI // 128          # 6 contraction chunks
NKT = S // 128           # 32 global key tiles per batch
NOWN = NKT // 2          # 16 own key tiles per core
QT = 512                 # query tile width
NQT = S // QT            # 8 query tiles
ORD = [0, 1, 3, 2, 7, 6, 5, 4]       # query-tile processing order
F32 = mybir.dt.float32
BF16 = mybir.dt.bfloat16
F8 = mybir.dt.float8e4
NWARM = 44               # dummy warmup matmuls (PE p-state bridge)

_prog_cache = {}


def j0_of(T):
    """First diagonal-region packed key tile for permuted query tile T."""
    return 4 * T if T < 4 else 4 * (T - 4)


def build_program():
    """Build + compile the single SPMD Bass program (identical on all cores)."""
    nc = bacc.Bacc("TRN2", target_bir_lowering=False, debug=False)

    # x^T relaid by the host to [partition, block, chunk, col]; block 0 is
    # stored [p, half, chunk, 256] so each half is one contiguous DMA.  Only
    # blocks 0-3 (own keys) are needed in bf16 (K/V projections); the Q
    # projection for blocks 1-7 runs in fp8 e4m3 DoubleRow (2x PE rate), fed
    # by a separate fp8 copy of x.
    xT = nc.declare_dram_parameter("xT", [128, 4 * NIC * QT], BF16,
                                   isOutput=False)
    xT8 = nc.declare_dram_parameter("xT8", [128, 7 * NIC * QT], F8,
                                    isOutput=False)
    # [Wk|Wv] then [Wq|Wq] (Wq duplicated so Q^T appears at partitions 0..63
    # AND 64..127), each relaid to [128, chunk, 128] contiguous per partition.
    wall = nc.declare_dram_parameter("wall", [128, 2 * NIC * 128], BF16,
                                     isOutput=False)
    w8 = nc.declare_dram_parameter("w8", [128, NIC * 128], F8, isOutput=False)
    # [mdiag | mpcol | ident(zero-padded)] as one [128, 320] block
    mall = nc.declare_dram_parameter("mall", [128, 320], BF16, isOutput=False)
    nd = nc.declare_dram_parameter("nd", [DO + 1, S], BF16, isOutput=True)

    with tile.TileContext(nc) as tc:
        with tc.tile_pool(name="consts", bufs=1) as consts, \
             tc.tile_pool(name="xpool", bufs=1) as xpool, \
             tc.tile_pool(name="qkv", bufs=1) as qkv, \
             tc.tile_pool(name="expp", bufs=10) as expp, \
             tc.tile_pool(name="ndst", bufs=4) as ndst, \
             tc.tile_pool(name="ps_sc", bufs=2, space="PSUM") as ps_sc, \
             tc.tile_pool(name="ps_pj", bufs=1, space="PSUM") as ps_pj, \
             tc.tile_pool(name="ps_ctx", bufs=1, space="PSUM") as ps_ctx:

            BW = NIC * QT  # 3072 cols per x block
            HB = BW // 2
            # ---- input DMAs, issued first thing.  Only sync/scalar/gpsimd
            # queues can start DMAs, and the sync/SP ring is ~10x slower than
            # the other two, so all bulk data rides scalar + gpsimd:
            #   scalar: xb0 half0, xb0 half1 (then free for the exp stream)
            #   gpsimd: twKV, twQ, [gate on h1], xb1, xb3, xb2, xb7, xb6,
            #           xb5, xb4  (+ nd outputs later)
            #   sync:   tm (small, not urgent)
            # The gate keeps xb1 from stealing HBM bandwidth from block 0.
            xb = [None] + [xpool.tile([128, BW], BF16, tag=f"xb_{cb}",
                                      name=f"xb_{cb}")
                           for cb in range(1, 4)]
            xb8 = [None] + [xpool.tile([128, NIC, QT], F8, tag=f"xb8_{cb}",
                                       name=f"xb8_{cb}")
                            for cb in range(1, 4)]
            xb8t = xpool.tile([128, 4, NIC, QT], F8, tag="xb8t", name="xb8t")
            xb8 += [xb8t[:, cb - 4] for cb in range(4, NQT)]
            xb0h = [xpool.tile([128, HB], BF16, tag=f"xb0h{h}", name=f"xb0h{h}")
                    for h in range(2)]
            twKV = consts.tile([128, NIC, 128], BF16, tag="twKV", name="twKV")
            twQ = consts.tile([128, NIC, 128], BF16, tag="twQ", name="twQ")
            tm = consts.tile([128, 320], BF16, tag="tm", name="tm")

            tw8 = consts.tile([128, NIC, 128], F8, tag="tw8", name="tw8")
            # Descriptor generation costs ~0.65us per DMA on the issuing
            # queue, so the 15 input DMAs are split: block 0 + block 1 ride
            # the scalar queue (free until the exp stream), the rest ride
            # gpsimd, both in consumption order.
            # The scalar ring gets ~2x bandwidth priority early, so block 0
            # rides it; weights lead the gpsimd ring.
            nc.gpsimd.dma_start(out=twKV, in_=wall[:, 0:NIC * 128])
            nc.scalar.dma_start(out=xb0h[0], in_=xT[:, 0:HB])
            nc.scalar.dma_start(out=xb0h[1], in_=xT[:, HB:BW])
            nc.gpsimd.dma_start(out=twQ, in_=wall[:, NIC * 128:2 * NIC * 128])
            nc.gpsimd.dma_start(out=tw8, in_=w8[:, :])
            nc.scalar.dma_start(out=xb8[1], in_=xT8[:, 0:BW])
            nc.scalar.dma_start(out=xb[1], in_=xT[:, BW:2 * BW])
            nc.sync.dma_start(out=tm, in_=mall[:, :])
            nc.gpsimd.dma_start(out=xb8[3], in_=xT8[:, 2 * BW:3 * BW])
            nc.gpsimd.dma_start(out=xb[2], in_=xT[:, 2 * BW:3 * BW])
            nc.gpsimd.dma_start(out=xb[3], in_=xT[:, 3 * BW:4 * BW])
            nc.gpsimd.dma_start(out=xb8[2], in_=xT8[:, 1 * BW:2 * BW])
            # blocks 4-7 (fp8 only, Q-proj) as ONE descriptor: descriptor
            # generation costs ~0.65us each on the issuing queue.
            nc.gpsimd.dma_start(out=xb8t, in_=xT8[:, 3 * BW:7 * BW])

            tmd = tm[:, 0:128]
            tmp = tm[:, 128:256]
            tid = tm[0:DO, 256:320]

            # ---- PE p-state bridge: dummy matmuls from program start until
            # the first x data lands, so the 3us continuous-execution ramp is
            # complete when real work starts.
            dum = consts.tile([128, 128], BF16, tag="dum", name="dum")
            nc.vector.memset(dum, 0.0)
            pdum = ps_sc.tile([128, 3 * QT], F32, tag="psc", name="psc")
            for _ in range(NWARM):
                nc.tensor.matmul(pdum[:, 0:128], dum, dum, start=True, stop=True)

            zsrc = consts.tile([DO, 1], F32, tag="zsrc", name="zsrc")
            nc.vector.memset(zsrc, 0.0)
            # Dummy exp pulls the ~1.3us ACT table load off the critical path.
            zexp = consts.tile([DO, 1], F32, tag="zexp", name="zexp")
            nc.scalar.activation(zexp, zsrc,
                                 mybir.ActivationFunctionType.Exp, scale=1.0)

            def xc(ic, cb):
                return xb[cb][:, ic * QT:(ic + 1) * QT]

            def xc0(half, ic):
                return xb0h[half][:, ic * 256:(ic + 1) * 256]

            # ---- projection state ----
            # kp[i]: K^T of key tile 2i at partitions 0..63, tile 2i+1 at
            # 64..127
            kps = [qkv.tile([128, 128], BF16, tag=f"kp_{i}", name=f"kp_{i}")
                   for i in range(NOWN // 2)]
            vts = [qkv.tile([DO, QT], BF16, tag=f"vt_{st}", name=f"vt_{st}")
                   for st in range(4)]
            qts = [qkv.tile([128, QT], BF16, tag=f"qt_{st}", name=f"qt_{st}")
                   for st in range(NQT)]
            # all V1 tiles in one buffer: [128 keys, key tile, 64 V cols + 1s]
            v1big = qkv.tile([128, NOWN, DO + 1], BF16, tag="v1big",
                             name="v1big")
            nc.vector.memset(v1big[:, :, DO:DO + 1], 1.0)

            def v1(j):
                return v1big[:, j, :]

            def kv_units(st):
                """K/V projection of own key column block st, as small PE
                units; copies split across Vector and GpSimd."""
                p1 = ps_pj.tile([128, QT], F32, tag="pspj", name="pspj")
                for ic in range(0, NIC, 2):
                    def mm2(ic=ic, p1=p1):
                        nc.tensor.matmul(p1, twKV[:, ic, :], xc(ic, st),
                                         start=(ic == 0), stop=False)
                        nc.tensor.matmul(p1, twKV[:, ic + 1, :], xc(ic + 1, st),
                                         start=False, stop=(ic + 1 == NIC - 1))
                    yield mm2

                def copies(p1=p1):
                    nc.vector.tensor_copy(vts[st], p1[DO:128, :])
                    for u in range(2):
                        kp = kps[2 * st + u]
                        nc.vector.tensor_copy(kp[0:DO, :],
                                              p1[0:DO, 256 * u:256 * u + 128])
                        nc.vector.tensor_copy(kp[DO:128, :],
                                              p1[0:DO, 256 * u + 128:256 * u + 256])
                yield copies

            def tr_unit(st, pre=None):
                """V transposes for block st -> v1big rows 4st..4st+3."""
                if pre is not None:
                    yield pre
                def transp():
                    pvq = ps_pj.tile([128, 4, DO], BF16, tag="pspj", name="pspj")
                    for r in range(4):
                        nc.tensor.transpose(pvq[:, r, :],
                                            vts[st][:, r * 128:r * 128 + 128],
                                            tid)
                    nc.vector.tensor_copy(v1big[:, 4 * st:4 * st + 4, 0:DO], pvq)
                yield transp

            def q_units(st):
                """Q^T (duplicated at partitions 0..63 / 64..127) for block
                st, in fp8 e4m3 DoubleRow mode: each matmul contracts TWO
                128-chunks at once at 2x PE rate."""
                p2 = ps_pj.tile([128, QT], F32, tag="pspj", name="pspj")
                for k in range(NIC // 2):
                    def mm(k=k, p2=p2):
                        nc.tensor.matmul(
                            p2, tw8[:, 2 * k:2 * k + 2, :],
                            xb8[st][:, 2 * k:2 * k + 2, :],
                            start=(k == 0), stop=(k == NIC // 2 - 1),
                            perf_mode=mybir.MatmulPerfMode.DoubleRow)
                    yield mm

                def qcopy(p2=p2):
                    nc.vector.tensor_copy(qts[st], p2)
                yield qcopy

            exp_scale = float(1.0 / np.sqrt(DO))

            def mm_sc(T, j, w, sp, off):
                """One K=64 scores matmul: key tile j x last w queries of tile
                T, into sp[:, off:off+w]. Row-group from j's parity."""
                kp = kps[j // 2]
                lo = DO * (j % 2)
                nc.tensor.matmul(sp[:, off:off + w], kp[lo:lo + DO, :],
                                 qts[T][lo:lo + DO, QT - w:QT],
                                 start=True, stop=True)

            class CtxDrain:
                """Phase B for a query tile, drained a few matmuls at a time
                via the filler queue so ctx work interleaves between the next
                tile's scores groups in the in-order PE queue."""

                def __init__(self, T, ctx_args):
                    self.T = T
                    self.nk = j0_of(T) + 4
                    self.args = ctx_args
                    self.i = 0
                    self.ctxp = ps_ctx.tile([DO + 1, QT], F32, tag="ctxp",
                                            name="ctxp")

                def drain(self, n):
                    while self.i < len(self.args) and n > 0:
                        j, et_ap, qlo, w = self.args[self.i]
                        nc.tensor.matmul(self.ctxp[:, qlo:QT], v1(j),
                                         et_ap[:, 0:w],
                                         start=(j == 0), stop=(j == self.nk - 1))
                        self.i += 1
                        n -= 1

                def finish(self):
                    self.drain(len(self.args))
                    ost = ndst.tile([DO + 1, QT], BF16, tag="ost", name="ost")
                    nc.vector.tensor_copy(ost, self.ctxp)
                    nc.gpsimd.dma_start(out=nd[:, self.T * QT:(self.T + 1) * QT],
                                        in_=ost)

            def emit_scores_full(T, j, cnt):
                sp = ps_sc.tile([128, 3 * QT], F32, tag="psc", name="psc")
                et = expp.tile([128, 3 * QT], BF16, tag="et", name="et")
                for u in range(cnt):
                    mm_sc(T, j + u, QT, sp, u * QT)
                return (sp, et, j, cnt)

            def emit_scores_band(T, j0):
                # diagonal band: all 4 tiles in ONE 3-bank tile / one exp:
                # r0 [0:512] bank1, r1 [512:896] bank2, r3 [896:1024] bank2,
                # r2 [1024:1280] bank3 (concurrent pairs hit distinct banks).
                sp = ps_sc.tile([128, 3 * QT], F32, tag="psc", name="psc")
                et = expp.tile([128, 3 * QT], BF16, tag="et", name="et")
                mm_sc(T, j0, QT, sp, 0)
                mm_sc(T, j0 + 1, 384, sp, QT)
                mm_sc(T, j0 + 2, 256, sp, 2 * QT)
                mm_sc(T, j0 + 3, 128, sp, QT + 384)
                return (sp, et, j0, -1)

            # ---- block-0 projections, per half: each half's matmul chain,
            # then its kp/vts/qts copies, start as soon as that half lands.
            p1 = ps_pj.tile([128, QT], F32, tag="pspj", name="pspj")
            p2 = ps_sc.tile([128, 3 * QT], F32, tag="psc", name="psc")
            for half in range(2):
                for ic in range(NIC):
                    nc.tensor.matmul(p1[:, half * 256:half * 256 + 256],
                                     twKV[:, ic, :], xc0(half, ic),
                                     start=(ic == 0), stop=(ic == NIC - 1))
                for ic in range(NIC):
                    nc.tensor.matmul(p2[:, half * 256:half * 256 + 256],
                                     twQ[:, ic, :], xc0(half, ic),
                                     start=(ic == 0), stop=(ic == NIC - 1))
                kp = kps[half]
                nc.vector.tensor_copy(kp[0:DO, :],
                                      p1[0:DO, 256 * half:256 * half + 128])
                nc.vector.tensor_copy(
                    kp[DO:128, :],
                    p1[0:DO, 256 * half + 128:256 * half + 256])
                nc.vector.tensor_copy(qts[0][:, 256 * half:256 * half + 256],
                                      p2[:, 256 * half:256 * half + 256])

            def b0_vts_copies():
                # vts[0] casts deferred off the band critical path: only the
                # tr0 transposes (pre first T0-ctx drain) need them.
                for half in range(2):
                    nc.vector.tensor_copy(
                        vts[0][:, 256 * half:256 * half + 256],
                        p1[DO:128, 256 * half:256 * half + 256])

            # ---- main loop ----
            # fillers: list of (deadline, seq, closure) proj units, kept
            # sorted (stable) by deadline.  Deadlines are GLOBAL SLOT ids:
            # each scores-group emission is one slot, numbered across the
            # whole kernel; a unit with deadline s is flushed before slot
            # s's scores are emitted.  Units of one generator share a
            # deadline, so stable sorting keeps each accumulation chain
            # contiguous in emission order (they share one PSUM buffer).
            fillers = []
            _seq = [0]

            def push_units(gen, dl):
                for u in gen:
                    fillers.append((dl, _seq[0], u))
                    _seq[0] += 1
                fillers.sort(key=lambda t: (t[0], t[1]))

            def run_fillers(n):
                k = 0
                while fillers and k < n:
                    fillers.pop(0)[2]()
                    k += 1

            def flush(s):
                while fillers and fillers[0][0] <= s:
                    fillers.pop(0)[2]()

            # slots per position: ceil(j0/3) full groups + 1 band
            SLOTS = [-(-j0_of(t) // 3) + 1 for t in ORD]
            START = [sum(SLOTS[:p]) for p in range(NQT)]   # first slot id
            BAND = [START[p] + SLOTS[p] - 1 for p in range(NQT)]

            # projection units per position: (st, kind, slot deadline).
            PROJ = {
                0: [(1, "q", START[1]), (0, "tr0", START[1]),
                    (1, "kv", BAND[1]), (1, "tr", START[2] + 2)],
                1: [(3, "q", START[2]), (2, "kv", START[2] + 2),
                    (3, "kv", BAND[2])],
                2: [(2, "q", START[3]), (2, "tr", START[3] + 1),
                    (3, "tr", START[3] + 2)],
                3: [(7, "q", START[4])],
                4: [(6, "q", START[5])],
                5: [(5, "q", START[6])],
                6: [(4, "q", START[7])],
                7: [],
            }

            pending = None  # CtxDrain from the previous iteration
            pre = None      # scores group already emitted via lookahead
            for pos in range(NQT):
                T = ORD[pos]
                j0 = j0_of(T)
                mask = tmd if T < 4 else tmp
                ctx_args = []   # (j, et_ap, qlo, w) drained via pending

                for st, kind, dl in PROJ[pos]:
                    if kind == "q":
                        push_units(q_units(st), dl)
                    elif kind == "kv":
                        push_units(kv_units(st), dl)
                    elif kind == "tr0":
                        push_units(tr_unit(st, pre=b0_vts_copies), dl)
                    else:
                        push_units(tr_unit(st), dl)
                # correctness: everything slot START[pos] (this position's
                # g0 scores / first ctx drains) depends on must be emitted.
                flush(START[pos])

                # group descriptors: (j, cnt) fulls in triples, then band
                descs = [(j, min(3, j0 - j)) for j in range(0, j0, 3)]
                descs.append((j0, -1))

                # pace leftovers + prev tile's ctx across the full-tile slots
                nslots = len(descs) - 1
                per_slot = -(-len(fillers) // nslots) if nslots else 0
                dn = (-(-len(pending.args) // nslots)
                      if pending is not None and nslots else 0)

                for gi, (j, cnt) in enumerate(descs):
                    is_band = cnt < 0
                    if gi == 0:
                        g = pre if pre is not None else (
                            emit_scores_band(T, j) if is_band
                            else emit_scores_full(T, j, cnt))
                        pre = None
                    # exp for this group
                    sp, et = g[0], g[1]
                    if is_band:
                        nc.scalar.activation(
                            et[:, 0:2 * QT + 256], sp[:, 0:2 * QT + 256],
                            mybir.ActivationFunctionType.Exp, scale=exp_scale)
                    else:
                        nc.scalar.activation(
                            et[:, 0:cnt * QT], sp[:, 0:cnt * QT],
                            mybir.ActivationFunctionType.Exp, scale=exp_scale)
                        for u in range(cnt):
                            ctx_args.append((j + u, et[:, u * QT:(u + 1) * QT],
                                             0, QT))
                    # lookahead: emit the NEXT group's scores now (after
                    # flushing exactly the units that group depends on), so
                    # the PE filler work of this slot can never starve the
                    # ACT stream.
                    if gi + 1 < len(descs):
                        jn, cn = descs[gi + 1]
                        flush(START[pos] + gi + 1)
                        if cn < 0:
                            g = emit_scores_band(T, jn)
                        else:
                            g = emit_scores_full(T, jn, cn)
                    elif pos < NQT - 1:
                        # boundary: next position's g0, after its deps
                        flush(START[pos + 1])
                        Tn = ORD[pos + 1]
                        j0n = j0_of(Tn)
                        pre = (emit_scores_band(Tn, j0n) if j0n == 0
                               else emit_scores_full(Tn, 0, min(3, j0n)))

                    if is_band:
                        # masks: split across Vector and GpSimd mid-kernel;
                        # Vector at the edges (gpsimd queue busy with DMA
                        # descriptors early; the tail ctx gates on them at
                        # the end).  Half-A deps (cols 0:128, 512:640) first.
                        nc.vector.tensor_mul(et[:, 0:128], et[:, 0:128], mask)
                        nc.gpsimd.tensor_mul(et[:, QT:QT + 128],
                                             et[:, QT:QT + 128], mask)
                        nc.vector.tensor_mul(et[:, QT + 384:2 * QT],
                                             et[:, QT + 384:2 * QT], mask)
                        nc.gpsimd.tensor_mul(et[:, 2 * QT:2 * QT + 128],
                                             et[:, 2 * QT:2 * QT + 128], mask)
                        ctx_args.append((j0, et[:, 0:QT], 0, QT))
                        ctx_args.append((j0 + 1, et[:, QT:QT + 384], 128, 384))
                        ctx_args.append((j0 + 2, et[:, 2 * QT:2 * QT + 256],
                                         256, 256))
                        ctx_args.append((j0 + 3, et[:, QT + 384:2 * QT],
                                         384, 128))
                        if pending is not None:
                            pending.finish()
                    else:
                        if pending is not None:
                            pending.drain(dn)
                        run_fillers(per_slot)

                if pos < NQT - 1:
                    pending = CtxDrain(T, ctx_args)
                else:
                    # ---- tail: drain the last tile's ctx in two column
                    # halves on PSUM buffers that are already free (ps_sc /
                    # ps_pj), so nothing waits on the previous tile's
                    # ps_ctx drain.  Masks pair across Vector+GpSimd, the
                    # half-A chain starts after just its two masks, and both
                    # output DMAs ride the gpsimd ring.
                    run_fillers(len(fillers))
                    eb = et  # band et tile (last group processed)
                    H = QT // 2
                    ctxpA = ps_sc.tile([DO + 1, H], F32, tag="psc",
                                       name="ctxpA")
                    # half A: output cols 0:256 <- tiles j0 (cols 0:256) and
                    # j0+1 (out cols 128:256 = its et cols 0:128)
                    nc.tensor.matmul(ctxpA, v1(0), eb[:, 0:H],
                                     start=True, stop=False)
                    nc.tensor.matmul(ctxpA[:, 128:H], v1(1),
                                     eb[:, QT:QT + 128],
                                     start=False, stop=True)
                    ostA = ndst.tile([DO + 1, H], BF16, tag="ost", name="ostA")
                    nc.vector.tensor_copy(ostA, ctxpA)
                    nc.gpsimd.dma_start(out=nd[:, T * QT:T * QT + H],
                                        in_=ostA)
                    # half B: output cols 256:512
                    ctxp2 = ps_pj.tile([DO + 1, H], F32, tag="pspj",
                                       name="ctxp2")
                    nc.tensor.matmul(ctxp2, v1(0), eb[:, H:QT],
                                     start=True, stop=False)
                    nc.tensor.matmul(ctxp2[:, 0:H], v1(1),
                                     eb[:, QT + 128:QT + 384],
                                     start=False, stop=False)
                    nc.tensor.matmul(ctxp2[:, 0:H], v1(2),
                                     eb[:, 2 * QT:2 * QT + 256],
                                     start=False, stop=False)
                    nc.tensor.matmul(ctxp2[:, 128:H], v1(3),
                                     eb[:, QT + 384:2 * QT],
                                     start=False, stop=True)
                    ostB = ndst.tile([DO + 1, H], BF16, tag="ost", name="ostB")
                    nc.vector.tensor_copy(ostB, ctxp2)
                    nc.gpsimd.dma_start(out=nd[:, T * QT + H:(T + 1) * QT],
                                        in_=ostB)

    nc.compile()
    return nc


def get_program():
    if "nc" not in _prog_cache:
        _prog_cache["nc"] = build_program()
    return _prog_cache["nc"]


def core_perm(parity):
    """Permuted-to-global column index map: own key tiles first, then other."""
    own = [g for g in range(NKT) if g % 2 == parity]
    other = [g for g in range(NKT) if g % 2 != parity]
    return np.concatenate([np.arange(g * 128, (g + 1) * 128)
                           for g in own + other])


def _to_bf16(a):
    from concourse import mybir as _mybir
    return np.ascontiguousarray(a.astype(_mybir.dt.np(_mybir.dt.bfloat16)))


def _to_f8(a):
    from concourse import mybir as _mybir
    return np.ascontiguousarray(a.astype(_mybir.dt.np(_mybir.dt.float8e4)))


def make_in_maps(x, Wq, Wk, Wv):
    x = np.asarray(x, dtype=np.float32)
    Wq = np.asarray(Wq, dtype=np.float32)
    Wk = np.asarray(Wk, dtype=np.float32)
    Wv = np.asarray(Wv, dtype=np.float32)
    wkv = np.concatenate([Wk, Wv], axis=1)                     # [768, 128]
    wqq = np.concatenate([Wq, Wq], axis=1)                     # [768, 128]
    wkv_r = wkv.reshape(NIC, 128, 128).transpose(1, 0, 2).reshape(128, -1)
    wqq_r = wqq.reshape(NIC, 128, 128).transpose(1, 0, 2).reshape(128, -1)
    wall = _to_bf16(np.concatenate([wkv_r, wqq_r], axis=1))    # [128, 1536]
    w8 = _to_f8(wqq_r)                                         # [128, 768]
    mdiag = np.triu(np.ones((128, 128), dtype=np.float32))
    identp = np.concatenate([np.eye(DO, dtype=np.float32),
                             np.zeros((128 - DO, DO), np.float32)], axis=0)
    in_maps = []
    perms = []
    for c in range(NCORES):
        b, par = c // 2, c % 2
        perm = core_perm(par)
        perms.append(perm)
        xTp = x[b].T[:, perm]                                  # [768, 4096]
        # [p, block, chunk, col] layout, contiguous per partition per block;
        # block 0 is stored [p, half, chunk, 256] so its two column halves
        # are each one contiguous DMA
        blocks = xTp.reshape(NIC, 128, NQT, QT).transpose(1, 2, 0, 3)
        b0 = (blocks[:, 0].reshape(128, NIC, 2, 256).transpose(0, 2, 1, 3)
              .reshape(128, NIC * QT))
        rest = blocks[:, 1:4].reshape(128, 3 * NIC * QT)
        xr = np.concatenate([b0, rest], axis=1)                # bf16: blk 0-3
        x8r = blocks[:, 1:].reshape(128, (NQT - 1) * NIC * QT)  # fp8: blk 1-7
        mpcol = np.full((128, 128), 1.0 - par, dtype=np.float32)
        mall = np.concatenate([mdiag, mpcol, identp], axis=1)  # [128, 320]
        in_maps.append({
            "xT": _to_bf16(xr), "xT8": _to_f8(x8r), "wall": wall, "w8": w8,
            "mall": _to_bf16(mall),
        })
    return in_maps, perms


def combine(results, perms):
    out = np.empty((B, S, DO), dtype=np.float32)
    for b in range(B):
        num = np.zeros((DO, S), dtype=np.float64)
        den = np.zeros((S,), dtype=np.float64)
        for c in (2 * b, 2 * b + 1):
            nd_c = results[c]["nd"].astype(np.float64)
            inv = np.empty(S, dtype=np.int64)
            inv[perms[c]] = np.arange(S)
            nd_g = nd_c[:, inv]
            num += nd_g[:DO]
            den += nd_g[DO]
        out[b] = (num / den).T.astype(np.float32)
    return out


def kernel(x, Wq, Wk, Wv):
    nc = get_program()
    in_maps, perms = make_in_maps(x, Wq, Wk, Wv)
    res = run_bass_kernel_spmd(nc, in_maps, list(range(NCORES)))
    return combine(res.results, perms)


# revision 24
# speedup vs baseline: 1.1435x; 1.0367x over previous
"""Causal attention (B=4, S=4096, D_IN=768, D_OUT=64) on 8 Trainium2 NeuronCores.

Sharding: core c handles batch b=c//2 and key-parity p=c%2 (the even or odd
128-wide key tiles of that batch). Every core computes, for ALL queries of its
batch, the unnormalized attention partials over its own key set:
    num[o, q] = sum_{k in own} exp(q.k/8) * V[k, o]
    den[q]    = sum_{k in own} exp(q.k/8)
The host sums the two partials per batch and normalizes: ctx = (num/den).T.
Causality is exact: key-tile work is skipped below the diagonal band and the
boundary blocks are masked with host-provided mask tiles.

Schedule (all bf16 on-chip, fp32 PSUM). Two serial resources matter:
the Scalar/ACT engine (~34us of exp streaming at 1.2GHz) and the PE
(~36us of matmul streaming at 2.4GHz -- but only 1.2GHz unless it has
run gap-free for 3us, so every stall is paid twice). The schedule:
 - x arrives via 4 DMA rings (scalar/gpsimd/vector/sync) with block 0 as
   two parallel halves and the weights split [KV|QQ] so the first
   projection matmul waits only on twKV + half 0.
 - dummy matmuls bridge the PE from program start to the first data
   arrival so the p-state ramp is complete when real work starts.
 - block-0 projections run per half as each half lands; kp/vts/qts
   copies are split between Vector and GpSimd.
 - the main loop emits one scores-group + one exp per "slot" and uses a
   filler queue (ctx drains of the previous tile, projection units for
   upcoming blocks) to pad the PE between slots, so the ACT stream
   never waits and the PE never idles.
 - scores matmuls run as CONCURRENT K=64 pairs on disjoint PE row
   groups (kp[i][0:64] = K^T of key tile 2i, kp[i][64:128] = tile 2i+1;
   Wq is sent duplicated so Q^T exists at partitions 0..63 AND 64..127).
 - ctx matmuls stay M=65 (64 V columns + ones column -> denominator
   free).
 - the last tile's ctx is drained in two column halves so the output
   copy+DMA of the first half overlaps the second half's matmuls.
"""
import numpy as np

import concourse.bass as bass
import concourse.bacc as bacc
import concourse.tile as tile
from concourse import mybir
from concourse.bass_utils import run_bass_kernel_spmd

B, S, DI, DO = 4, 4096, 768, 64
NCORES = 8
NIC = DI // 128          # 6 contraction chunks
NKT = S // 128           # 32 global key tiles per batch
NOWN = NKT // 2          # 16 own key tiles per core
QT = 512                 # query tile width
NQT = S // QT            # 8 query tiles
ORD = [0, 1, 3, 2, 7, 6, 5, 4]       # query-tile processing order
F32 = mybir.dt.float32
BF16 = mybir.dt.bfloat16
F8 = mybir.dt.float8e4
NWARM = 44               # dummy warmup matmuls (PE p-state bridge)

_prog_cache = {}


def j0_of(T):
    """First diagonal-region packed key tile for permuted query tile T."""
    return 4 * T if T < 4 else 4 * (T - 4)


def build_program():
    """Build + compile the single SPMD Bass program (identical on all cores)."""
    nc = bacc.Bacc("TRN2", target_bir_lowering=False, debug=False)

    # x^T relaid by the host to [partition, block, chunk, col]; block 0 is
    # stored [p, half, chunk, 256] so each half is one contiguous DMA.  Only
    # blocks 0-3 (own keys) are needed in bf16 (K/V projections); the Q
    # projection for blocks 1-7 runs in fp8 e4m3 DoubleRow (2x PE rate), fed
    # by a separate fp8 copy of x.
    xT = nc.declare_dram_parameter("xT", [128, 4 * NIC * QT], BF16,
                                   isOutput=False)
    xT8 = nc.declare_dram_parameter("xT8", [128, 7 * NIC * QT], F8,
                                    isOutput=False)
    # [Wk|Wv] then [Wq|Wq] (Wq duplicated so Q^T appears at partitions 0..63
    # AND 64..127), each relaid to [128, chunk, 128] contiguous per partition.
    wall = nc.declare_dram_parameter("wall", [128, 2 * NIC * 128], BF16,
                                     isOutput=False)
    w8 = nc.declare_dram_parameter("w8", [128, NIC * 128], F8, isOutput=False)
    # [mdiag | mpcol | ident(zero-padded)] as one [128, 320] block
    mall = nc.declare_dram_parameter("mall", [128, 320], BF16, isOutput=False)
    nd = nc.declare_dram_parameter("nd", [DO + 1, S], BF16, isOutput=True)

    with tile.TileContext(nc) as tc:
        with tc.tile_pool(name="consts", bufs=1) as consts, \
             tc.tile_pool(name="xpool", bufs=1) as xpool, \
             tc.tile_pool(name="qkv", bufs=1) as qkv, \
             tc.tile_pool(name="expp", bufs=10) as expp, \
             tc.tile_pool(name="ndst", bufs=4) as ndst, \
             tc.tile_pool(name="ps_sc", bufs=2, space="PSUM") as ps_sc, \
             tc.tile_pool(name="ps_pj", bufs=1, space="PSUM") as ps_pj, \
             tc.tile_pool(name="ps_ctx", bufs=1, space="PSUM") as ps_ctx:

            BW = NIC * QT  # 3072 cols per x block
            HB = BW // 2
            # ---- input DMAs, issued first thing.  Only sync/scalar/gpsimd
            # queues can start DMAs, and the sync/SP ring is ~10x slower than
            # the other two, so all bulk data rides scalar + gpsimd:
            #   scalar: xb0 half0, xb0 half1 (then free for the exp stream)
            #   gpsimd: twKV, twQ, [gate on h1], xb1, xb3, xb2, xb7, xb6,
            #           xb5, xb4  (+ nd outputs later)
            #   sync:   tm (small, not urgent)
            # The gate keeps xb1 from stealing HBM bandwidth from block 0.
            xb = [None] + [xpool.tile([128, BW], BF16, tag=f"xb_{cb}",
                                      name=f"xb_{cb}")
                           for cb in range(1, 4)]
            xb8 = [None] + [xpool.tile([128, NIC, QT], F8, tag=f"xb8_{cb}",
                                       name=f"xb8_{cb}")
                            for cb in range(1, 4)]
            xb8t = xpool.tile([128, 4, NIC, QT], F8, tag="xb8t", name="xb8t")
            xb8 += [xb8t[:, cb - 4] for cb in range(4, NQT)]
            xb0h = [xpool.tile([128, HB], BF16, tag=f"xb0h{h}", name=f"xb0h{h}")
                    for h in range(2)]
            twKV = consts.tile([128, NIC, 128], BF16, tag="twKV", name="twKV")
            twQ = consts.tile([128, NIC, 128], BF16, tag="twQ", name="twQ")
            tm = consts.tile([128, 320], BF16, tag="tm", name="tm")

            tw8 = consts.tile([128, NIC, 128], F8, tag="tw8", name="tw8")
            # Descriptor generation costs ~0.65us per DMA on the issuing
            # queue, so the 15 input DMAs are split: block 0 + block 1 ride
            # the scalar queue (free until the exp stream), the rest ride
            # gpsimd, both in consumption order.
            # The scalar ring gets ~2x bandwidth priority early, so block 0
            # rides it; weights lead the gpsimd ring.
            nc.gpsimd.dma_start(out=twKV, in_=wall[:, 0:NIC * 128])
            nc.scalar.dma_start(out=xb0h[0], in_=xT[:, 0:HB])
            nc.scalar.dma_start(out=xb0h[1], in_=xT[:, HB:BW])
            nc.gpsimd.dma_start(out=twQ, in_=wall[:, NIC * 128:2 * NIC * 128])
            nc.gpsimd.dma_start(out=tw8, in_=w8[:, :])
            nc.scalar.dma_start(out=xb8[1], in_=xT8[:, 0:BW])
            nc.scalar.dma_start(out=xb[1], in_=xT[:, BW:2 * BW])
            nc.sync.dma_start(out=tm, in_=mall[:, :])
            nc.gpsimd.dma_start(out=xb8[3], in_=xT8[:, 2 * BW:3 * BW])
            nc.gpsimd.dma_start(out=xb[2], in_=xT[:, 2 * BW:3 * BW])
            nc.gpsimd.dma_start(out=xb[3], in_=xT[:, 3 * BW:4 * BW])
            nc.gpsimd.dma_start(out=xb8[2], in_=xT8[:, 1 * BW:2 * BW])
            # blocks 4-7 (fp8 only, Q-proj) as ONE descriptor: descriptor
            # generation costs ~0.65us each on the issuing queue.
            nc.gpsimd.dma_start(out=xb8t, in_=xT8[:, 3 * BW:7 * BW])

            tmd = tm[:, 0:128]
            tmp = tm[:, 128:256]
            tid = tm[0:DO, 256:320]

            # ---- PE p-state bridge: dummy matmuls from program start until
            # the first x data lands, so the 3us continuous-execution ramp is
            # complete when real work starts.
            dum = consts.tile([128, 128], BF16, tag="dum", name="dum")
            nc.vector.memset(dum, 0.0)
            pdum = ps_sc.tile([128, 3 * QT], F32, tag="psc", name="psc")
            for _ in range(NWARM):
                nc.tensor.matmul(pdum[:, 0:128], dum, dum, start=True, stop=True)

            zsrc = consts.tile([DO, 1], F32, tag="zsrc", name="zsrc")
            nc.vector.memset(zsrc, 0.0)
            # Dummy exp pulls the ~1.3us ACT table load off the critical path.
            zexp = consts.tile([DO, 1], F32, tag="zexp", name="zexp")
            nc.scalar.activation(zexp, zsrc,
                                 mybir.ActivationFunctionType.Exp, scale=1.0)

            def xc(ic, cb):
                return xb[cb][:, ic * QT:(ic + 1) * QT]

            def xc0(half, ic):
                return xb0h[half][:, ic * 256:(ic + 1) * 256]

            # ---- projection state ----
            # kp[i]: K^T of key tile 2i at partitions 0..63, tile 2i+1 at
            # 64..127
            kps = [qkv.tile([128, 128], BF16, tag=f"kp_{i}", name=f"kp_{i}")
                   for i in range(NOWN // 2)]
            vts = [qkv.tile([DO, QT], BF16, tag=f"vt_{st}", name=f"vt_{st}")
                   for st in range(4)]
            qts = [qkv.tile([128, QT], BF16, tag=f"qt_{st}", name=f"qt_{st}")
                   for st in range(NQT)]
            # all V1 tiles in one buffer: [128 keys, key tile, 64 V cols + 1s]
            v1big = qkv.tile([128, NOWN, DO + 1], BF16, tag="v1big",
                             name="v1big")
            nc.vector.memset(v1big[:, :, DO:DO + 1], 1.0)

            def v1(j):
                return v1big[:, j, :]

            def kv_units(st):
                """K/V projection of own key column block st, as small PE
                units; copies split across Vector and GpSimd."""
                p1 = ps_pj.tile([128, QT], F32, tag="pspj", name="pspj")
                for ic in range(0, NIC, 2):
                    def mm2(ic=ic, p1=p1):
                        nc.tensor.matmul(p1, twKV[:, ic, :], xc(ic, st),
                                         start=(ic == 0), stop=False)
                        nc.tensor.matmul(p1, twKV[:, ic + 1, :], xc(ic + 1, st),
                                         start=False, stop=(ic + 1 == NIC - 1))
                    yield mm2

                def copies(p1=p1):
                    nc.vector.tensor_copy(vts[st], p1[DO:128, :])
                    for u in range(2):
                        kp = kps[2 * st + u]
                        nc.vector.tensor_copy(kp[0:DO, :],
                                              p1[0:DO, 256 * u:256 * u + 128])
                        nc.vector.tensor_copy(kp[DO:128, :],
                                              p1[0:DO, 256 * u + 128:256 * u + 256])
                yield copies

            def tr_unit(st, pre=None):
                """V transposes for block st -> v1big rows 4st..4st+3."""
                if pre is not None:
                    yield pre
                def transp():
                    pvq = ps_pj.tile([128, 4, DO], BF16, tag="pspj", name="pspj")
                    for r in range(4):
                        nc.tensor.transpose(pvq[:, r, :],
                                            vts[st][:, r * 128:r * 128 + 128],
                                            tid)
                    nc.vector.tensor_copy(v1big[:, 4 * st:4 * st + 4, 0:DO], pvq)
                yield transp

            def q_units(st):
                """Q^T (duplicated at partitions 0..63 / 64..127) for block
                st, in fp8 e4m3 DoubleRow mode: each matmul contracts TWO
                128-chunks at once at 2x PE rate."""
                p2 = ps_pj.tile([128, QT], F32, tag="pspj", name="pspj")
                for k in range(NIC // 2):
                    def mm(k=k, p2=p2):
                        nc.tensor.matmul(
                            p2, tw8[:, 2 * k:2 * k + 2, :],
                            xb8[st][:, 2 * k:2 * k + 2, :],
                            start=(k == 0), stop=(k == NIC // 2 - 1),
                            perf_mode=mybir.MatmulPerfMode.DoubleRow)
                    yield mm

                def qcopy(p2=p2):
                    nc.vector.tensor_copy(qts[st], p2)
                yield qcopy

            exp_scale = float(1.0 / np.sqrt(DO))

            def mm_sc(T, j, w, sp, off):
                """One K=64 scores matmul: key tile j x last w queries of tile
                T, into sp[:, off:off+w]. Row-group from j's parity."""
                kp = kps[j // 2]
                lo = DO * (j % 2)
                nc.tensor.matmul(sp[:, off:off + w], kp[lo:lo + DO, :],
                                 qts[T][lo:lo + DO, QT - w:QT],
                                 start=True, stop=True)

            class CtxDrain:
                """Phase B for a query tile, drained a few matmuls at a time
                via the filler queue so ctx work interleaves between the next
                tile's scores groups in the in-order PE queue."""

                def __init__(self, T, ctx_args):
                    self.T = T
                    self.nk = j0_of(T) + 4
                    self.args = ctx_args
                    self.i = 0
                    self.ctxp = ps_ctx.tile([DO + 1, QT], F32, tag="ctxp",
                                            name="ctxp")

                def drain(self, n):
                    while self.i < len(self.args) and n > 0:
                        j, et_ap, qlo, w = self.args[self.i]
                        nc.tensor.matmul(self.ctxp[:, qlo:QT], v1(j),
                                         et_ap[:, 0:w],
                                         start=(j == 0), stop=(j == self.nk - 1))
                        self.i += 1
                        n -= 1

                def finish(self):
                    self.drain(len(self.args))
                    ost = ndst.tile([DO + 1, QT], BF16, tag="ost", name="ost")
                    nc.vector.tensor_copy(ost, self.ctxp)
                    nc.gpsimd.dma_start(out=nd[:, self.T * QT:(self.T + 1) * QT],
                                        in_=ost)

            def emit_scores_full(T, j, cnt):
                sp = ps_sc.tile([128, 3 * QT], F32, tag="psc", name="psc")
                et = expp.tile([128, 3 * QT], BF16, tag="et", name="et")
                for u in range(cnt):
                    mm_sc(T, j + u, QT, sp, u * QT)
                return (sp, et, j, cnt)

            def emit_scores_band(T, j0):
                # diagonal band: all 4 tiles in ONE 3-bank tile / one exp:
                # r0 [0:512] bank1, r1 [512:896] bank2, r3 [896:1024] bank2,
                # r2 [1024:1280] bank3 (concurrent pairs hit distinct banks).
                sp = ps_sc.tile([128, 3 * QT], F32, tag="psc", name="psc")
                et = expp.tile([128, 3 * QT], BF16, tag="et", name="et")
                mm_sc(T, j0, QT, sp, 0)
                mm_sc(T, j0 + 1, 384, sp, QT)
                mm_sc(T, j0 + 2, 256, sp, 2 * QT)
                mm_sc(T, j0 + 3, 128, sp, QT + 384)
                return (sp, et, j0, -1)

            # ---- block-0 projections, per half: each half's matmul chain,
            # then its kp/vts/qts copies, start as soon as that half lands.
            p1 = ps_pj.tile([128, QT], F32, tag="pspj", name="pspj")
            p2 = ps_sc.tile([128, 3 * QT], F32, tag="psc", name="psc")
            for half in range(2):
                for ic in range(NIC):
                    nc.tensor.matmul(p1[:, half * 256:half * 256 + 256],
                                     twKV[:, ic, :], xc0(half, ic),
                                     start=(ic == 0), stop=(ic == NIC - 1))
                for ic in range(NIC):
                    nc.tensor.matmul(p2[:, half * 256:half * 256 + 256],
                                     twQ[:, ic, :], xc0(half, ic),
                                     start=(ic == 0), stop=(ic == NIC - 1))
                kp = kps[half]
                nc.vector.tensor_copy(kp[0:DO, :],
                                      p1[0:DO, 256 * half:256 * half + 128])
                nc.vector.tensor_copy(
                    kp[DO:128, :],
                    p1[0:DO, 256 * half + 128:256 * half + 256])
                nc.vector.tensor_copy(qts[0][:, 256 * half:256 * half + 256],
                                      p2[:, 256 * half:256 * half + 256])

            def b0_vts_copies():
                # vts[0] casts deferred off the band critical path: only the
                # tr0 transposes (pre first T0-ctx drain) need them.
                for half in range(2):
                    nc.vector.tensor_copy(
                        vts[0][:, 256 * half:256 * half + 256],
                        p1[DO:128, 256 * half:256 * half + 256])

            # ---- main loop ----
            # fillers: list of (deadline, seq, closure) proj units, kept
            # sorted (stable) by deadline.  Deadlines are GLOBAL SLOT ids:
            # each scores-group emission is one slot, numbered across the
            # whole kernel; a unit with deadline s is flushed before slot
            # s's scores are emitted.  Units of one generator share a
            # deadline, so stable sorting keeps each accumulation chain
            # contiguous in emission order (they share one PSUM buffer).
            fillers = []
            _seq = [0]

            def push_units(gen, dl):
                for u in gen:
                    fillers.append((dl, _seq[0], u))
                    _seq[0] += 1
                fillers.sort(key=lambda t: (t[0], t[1]))

            def run_fillers(n):
                k = 0
                while fillers and k < n:
                    fillers.pop(0)[2]()
                    k += 1

            def flush(s):
                while fillers and fillers[0][0] <= s:
                    fillers.pop(0)[2]()

            # slots per position: ceil(j0/3) full groups + 1 band
            SLOTS = [-(-j0_of(t) // 3) + 1 for t in ORD]
            START = [sum(SLOTS[:p]) for p in range(NQT)]   # first slot id
            BAND = [START[p] + SLOTS[p] - 1 for p in range(NQT)]

            # projection units per position: (st, kind, slot deadline).
            PROJ = {
                0: [(1, "q", START[1]), (0, "tr0", START[1]),
                    (1, "kv", BAND[1]), (1, "tr", START[2] + 2)],
                1: [(3, "q", START[2]), (2, "kv", START[2] + 2),
                    (3, "kv", BAND[2])],
                2: [(2, "q", START[3]), (2, "tr", START[3] + 1),
                    (3, "tr", START[3] + 2)],
                3: [(7, "q", START[4])],
                4: [(6, "q", START[5])],
                5: [(5, "q", START[6])],
                6: [(4, "q", START[7])],
                7: [],
            }

            pending = None  # CtxDrain from the previous iteration
            pre = None      # scores group already emitted via lookahead
            for pos in range(NQT):
                T = ORD[pos]
                j0 = j0_of(T)
                mask = tmd if T < 4 else tmp
                ctx_args = []   # (j, et_ap, qlo, w) drained via pending

                for st, kind, dl in PROJ[pos]:
                    if kind == "q":
                        push_units(q_units(st), dl)
                    elif kind == "kv":
                        push_units(kv_units(st), dl)
                    elif kind == "tr0":
                        push_units(tr_unit(st, pre=b0_vts_copies), dl)
                    else:
                        push_units(tr_unit(st), dl)
                # correctness: everything slot START[pos] (this position's
                # g0 scores / first ctx drains) depends on must be emitted.
                flush(START[pos])

                # group descriptors: (j, cnt) fulls in triples, then band
                descs = [(j, min(3, j0 - j)) for j in range(0, j0, 3)]
                descs.append((j0, -1))

                # pace leftovers + prev tile's ctx across the full-tile slots
                nslots = len(descs) - 1
                per_slot = -(-len(fillers) // nslots) if nslots else 0
                dn = (-(-len(pending.args) // nslots)
                      if pending is not None and nslots else 0)
                if pos == NQT - 1 and pending is not None:
                    # band-only position: no slots would drain the previous
                    # tile's ctx before the tail, so drain it here -- its
                    # exps (and all but the band masks) are long done.
                    pending.drain(len(pending.args) - 4)

                for gi, (j, cnt) in enumerate(descs):
                    is_band = cnt < 0
                    if gi == 0:
                        g = pre if pre is not None else (
                            emit_scores_band(T, j) if is_band
                            else emit_scores_full(T, j, cnt))
                        pre = None
                    # exp for this group
                    sp, et = g[0], g[1]
                    if is_band:
                        nc.scalar.activation(
                            et[:, 0:2 * QT + 256], sp[:, 0:2 * QT + 256],
                            mybir.ActivationFunctionType.Exp, scale=exp_scale)
                    else:
                        nc.scalar.activation(
                            et[:, 0:cnt * QT], sp[:, 0:cnt * QT],
                            mybir.ActivationFunctionType.Exp, scale=exp_scale)
                        for u in range(cnt):
                            ctx_args.append((j + u, et[:, u * QT:(u + 1) * QT],
                                             0, QT))
                    # lookahead: emit the NEXT group's scores now (after
                    # flushing exactly the units that group depends on), so
                    # the PE filler work of this slot can never starve the
                    # ACT stream.
                    if gi + 1 < len(descs):
                        jn, cn = descs[gi + 1]
                        flush(START[pos] + gi + 1)
                        if cn < 0:
                            g = emit_scores_band(T, jn)
                        else:
                            g = emit_scores_full(T, jn, cn)
                    elif pos < NQT - 1:
                        # boundary: next position's g0, after its deps
                        flush(START[pos + 1])
                        Tn = ORD[pos + 1]
                        j0n = j0_of(Tn)
                        pre = (emit_scores_band(Tn, j0n) if j0n == 0
                               else emit_scores_full(Tn, 0, min(3, j0n)))

                    if is_band:
                        # masks: split across Vector and GpSimd mid-kernel;
                        # Vector at the edges (gpsimd queue busy with DMA
                        # descriptors early; the tail ctx gates on them at
                        # the end).  Half-A deps (cols 0:128, 512:640) first.
                        nc.vector.tensor_mul(et[:, 0:128], et[:, 0:128], mask)
                        nc.gpsimd.tensor_mul(et[:, QT:QT + 128],
                                             et[:, QT:QT + 128], mask)
                        nc.vector.tensor_mul(et[:, QT + 384:2 * QT],
                                             et[:, QT + 384:2 * QT], mask)
                        nc.gpsimd.tensor_mul(et[:, 2 * QT:2 * QT + 128],
                                             et[:, 2 * QT:2 * QT + 128], mask)
                        ctx_args.append((j0, et[:, 0:QT], 0, QT))
                        ctx_args.append((j0 + 1, et[:, QT:QT + 384], 128, 384))
                        ctx_args.append((j0 + 2, et[:, 2 * QT:2 * QT + 256],
                                         256, 256))
                        ctx_args.append((j0 + 3, et[:, QT + 384:2 * QT],
                                         384, 128))
                        if pending is not None and pos < NQT - 1:
                            pending.finish()
                    else:
                        if pending is not None:
                            pending.drain(dn)
                        run_fillers(per_slot)

                if pos < NQT - 1:
                    pending = CtxDrain(T, ctx_args)
                else:
                    # ---- tail: drain the last tile's ctx in two column
                    # halves on PSUM buffers that are already free (ps_sc /
                    # ps_pj), so nothing waits on the previous tile's
                    # ps_ctx drain.  Masks pair across Vector+GpSimd, the
                    # half-A chain starts after just its two masks, and both
                    # output DMAs ride the gpsimd ring.
                    run_fillers(len(fillers))
                    eb = et  # band et tile (last group processed)
                    H = QT // 2
                    ctxpA = ps_sc.tile([DO + 1, H], F32, tag="psc",
                                       name="ctxpA")
                    # half A: output cols 0:256 <- tiles j0 (cols 0:256) and
                    # j0+1 (out cols 128:256 = its et cols 0:128)
                    nc.tensor.matmul(ctxpA, v1(0), eb[:, 0:H],
                                     start=True, stop=False)
                    nc.tensor.matmul(ctxpA[:, 128:H], v1(1),
                                     eb[:, QT:QT + 128],
                                     start=False, stop=True)
                    ostA = ndst.tile([DO + 1, H], BF16, tag="ost", name="ostA")
                    nc.vector.tensor_copy(ostA, ctxpA)
                    nc.gpsimd.dma_start(out=nd[:, T * QT:T * QT + H],
                                        in_=ostA)
                    # previous tile's remaining (band) ctx + output, behind
                    # the tail's half-A on both queues
                    if pending is not None:
                        pending.finish()
                    # half B: output cols 256:512
                    ctxp2 = ps_pj.tile([DO + 1, H], F32, tag="pspj",
                                       name="ctxp2")
                    nc.tensor.matmul(ctxp2, v1(0), eb[:, H:QT],
                                     start=True, stop=False)
                    nc.tensor.matmul(ctxp2[:, 0:H], v1(1),
                                     eb[:, QT + 128:QT + 384],
                                     start=False, stop=False)
                    nc.tensor.matmul(ctxp2[:, 0:H], v1(2),
                                     eb[:, 2 * QT:2 * QT + 256],
                                     start=False, stop=False)
                    nc.tensor.matmul(ctxp2[:, 128:H], v1(3),
                                     eb[:, QT + 384:2 * QT],
                                     start=False, stop=True)
                    ostB = ndst.tile([DO + 1, H], BF16, tag="ost", name="ostB")
                    nc.vector.tensor_copy(ostB, ctxp2)
                    nc.gpsimd.dma_start(out=nd[:, T * QT + H:(T + 1) * QT],
                                        in_=ostB)

    nc.compile()
    return nc


def get_program():
    if "nc" not in _prog_cache:
        _prog_cache["nc"] = build_program()
    return _prog_cache["nc"]


def core_perm(parity):
    """Permuted-to-global column index map: own key tiles first, then other."""
    own = [g for g in range(NKT) if g % 2 == parity]
    other = [g for g in range(NKT) if g % 2 != parity]
    return np.concatenate([np.arange(g * 128, (g + 1) * 128)
                           for g in own + other])


def _to_bf16(a):
    from concourse import mybir as _mybir
    return np.ascontiguousarray(a.astype(_mybir.dt.np(_mybir.dt.bfloat16)))


def _to_f8(a):
    from concourse import mybir as _mybir
    return np.ascontiguousarray(a.astype(_mybir.dt.np(_mybir.dt.float8e4)))


def make_in_maps(x, Wq, Wk, Wv):
    x = np.asarray(x, dtype=np.float32)
    Wq = np.asarray(Wq, dtype=np.float32)
    Wk = np.asarray(Wk, dtype=np.float32)
    Wv = np.asarray(Wv, dtype=np.float32)
    wkv = np.concatenate([Wk, Wv], axis=1)                     # [768, 128]
    wqq = np.concatenate([Wq, Wq], axis=1)                     # [768, 128]
    wkv_r = wkv.reshape(NIC, 128, 128).transpose(1, 0, 2).reshape(128, -1)
    wqq_r = wqq.reshape(NIC, 128, 128).transpose(1, 0, 2).reshape(128, -1)
    wall = _to_bf16(np.concatenate([wkv_r, wqq_r], axis=1))    # [128, 1536]
    w8 = _to_f8(wqq_r)                                         # [128, 768]
    mdiag = np.triu(np.ones((128, 128), dtype=np.float32))
    identp = np.concatenate([np.eye(DO, dtype=np.float32),
                             np.zeros((128 - DO, DO), np.float32)], axis=0)
    in_maps = []
    perms = []
    for c in range(NCORES):
        b, par = c // 2, c % 2
        perm = core_perm(par)
        perms.append(perm)
        xTp = x[b].T[:, perm]                                  # [768, 4096]
        # [p, block, chunk, col] layout, contiguous per partition per block;
        # block 0 is stored [p, half, chunk, 256] so its two column halves
        # are each one contiguous DMA
        blocks = xTp.reshape(NIC, 128, NQT, QT).transpose(1, 2, 0, 3)
        b0 = (blocks[:, 0].reshape(128, NIC, 2, 256).transpose(0, 2, 1, 3)
              .reshape(128, NIC * QT))
        rest = blocks[:, 1:4].reshape(128, 3 * NIC * QT)
        xr = np.concatenate([b0, rest], axis=1)                # bf16: blk 0-3
        x8r = blocks[:, 1:].reshape(128, (NQT - 1) * NIC * QT)  # fp8: blk 1-7
        mpcol = np.full((128, 128), 1.0 - par, dtype=np.float32)
        mall = np.concatenate([mdiag, mpcol, identp], axis=1)  # [128, 320]
        in_maps.append({
            "xT": _to_bf16(xr), "xT8": _to_f8(x8r), "wall": wall, "w8": w8,
            "mall": _to_bf16(mall),
        })
    return in_maps, perms


def combine(results, perms):
    out = np.empty((B, S, DO), dtype=np.float32)
    for b in range(B):
        num = np.zeros((DO, S), dtype=np.float64)
        den = np.zeros((S,), dtype=np.float64)
        for c in (2 * b, 2 * b + 1):
            nd_c = results[c]["nd"].astype(np.float64)
            inv = np.empty(S, dtype=np.int64)
            inv[perms[c]] = np.arange(S)
            nd_g = nd_c[:, inv]
            num += nd_g[:DO]
            den += nd_g[DO]
        out[b] = (num / den).T.astype(np.float32)
    return out


def kernel(x, Wq, Wk, Wv):
    nc = get_program()
    in_maps, perms = make_in_maps(x, Wq, Wk, Wv)
    res = run_bass_kernel_spmd(nc, in_maps, list(range(NCORES)))
    return combine(res.results, perms)
